# revision 32
# baseline (speedup 1.0000x reference)
"""Multi-head attention (B=4, S=2048, D=512, H=8) on 8 trn2 NeuronCores.

Sharding: core c -> batch b = c//2, feature-slice g = c%2 (256 features =
4 heads).  Each core computes Q/K/V projections for its 4 heads, a
flash-style streaming softmax-attention (no S x S materialization in HBM),
and a partial output projection through its 256-column slice of wo.  The
host sums the two partials per batch and adds the (bo + bv @ wo.T) constant.

v4: single software-pipelined loop of 128 (qc512, jp, kc) iterations.
Each iteration runs the head PAIR of jp: two row-tiled QK matmuls (heads
at PE rows 0:64 / 64:128 overlap in the array), ONE [128,1024] EXP
covering both heads, and two [65,512] AV accumulations.  K/Q/V projection
chunks and wo chains are emitted as PE filler at scheduled iterations;
DMAs issue in priority order on the sync + gpsimd sequencers.  PSUM:
2x[128,1024] QK double-buffer + 2x[65,512] AV + 2x[128,512] proj = 8 banks.

All activations stay "transposed" ([feature, seq]) on device:
  QT = (wq/8 @ x_q.T + bq/8), KT = wk @ x_k.T + bk          [256, 2048]
  attT_h = K_h @ Q_h.T  (k on partitions, q free)            [2048, 2048]
  E = exp(attT)  (no max subtraction: logits ~ N(0,1))
  [outT; denom] = [V_h | 1].T @ E  (ones column -> denominators)
  outT_norm = outT * (1/denom broadcast)                     [256, 2048]
  out_partial = outT_norm.T @ (wo slice)                     [2048, 512]

Numerics: QK^T in fp32 psum; E / V / weights / partial outputs in fp16.
"""

import os
import sys

for _p in ("/opt/trn_rl_repo", "/root/.axon_site/_ro/trn_rl_repo"):
    if os.path.isdir(_p) and _p not in sys.path:
        sys.path.append(_p)

import numpy as np

import concourse.bass as bass
import concourse.bacc as bacc
import concourse.tile as tile
import concourse.mybir as mybir
from concourse.bass import ts
from concourse.bass_utils import run_bass_kernel_spmd

F32 = mybir.dt.float32
BF16 = mybir.dt.float16
AF = mybir.ActivationFunctionType

B, S, D = 4, 2048, 512
NH, DK = 8, 64
FS = 256           # features per core (4 heads)
NJ = 4             # local heads
QW = 512           # q-chunk per sweep
NQC = S // QW      # 4
NKC = S // 128     # 16 k-chunks
NDC = D // 128     # 4 contraction chunks for projections
NSW = NQC * 2      # 8 sweeps (qc, jp)
NIT = NSW * NKC    # 128 pipeline iterations

_cache = {}


def build_nc():
    nc = bacc.Bacc("TRN2", target_bir_lowering=False, debug=False)

    xq_d = nc.dram_tensor("xq", [D, S], BF16, kind="ExternalInput")
    xk_d = nc.dram_tensor("xk", [D, S], BF16, kind="ExternalInput")
    xv_d = nc.dram_tensor("xv", [D, S], BF16, kind="ExternalInput")
    wq_d = nc.dram_tensor("wq", [D, FS], BF16, kind="ExternalInput")
    wk_d = nc.dram_tensor("wk", [D, FS], BF16, kind="ExternalInput")
    wv_d = nc.dram_tensor("wv", [D, NJ * 65], BF16, kind="ExternalInput")
    wo_d = nc.dram_tensor("wo", [FS, D], BF16, kind="ExternalInput")
    bq_d = nc.dram_tensor("bq", [128, 2], F32, kind="ExternalInput")
    bk_d = nc.dram_tensor("bk", [128, 2], F32, kind="ExternalInput")
    mask_d = nc.dram_tensor("mask", [128, NJ * 65], F32, kind="ExternalInput")
    out_d = nc.dram_tensor("out", [S, D], BF16, kind="ExternalOutput")

    with tile.TileContext(nc) as tc:
        from contextlib import ExitStack
        with ExitStack() as ctx:
            consts = ctx.enter_context(tc.tile_pool(name="consts", bufs=1))
            xpool = ctx.enter_context(tc.tile_pool(name="x", bufs=12))
            acts = ctx.enter_context(tc.tile_pool(name="acts", bufs=1))
            epool = ctx.enter_context(tc.tile_pool(name="expatt", bufs=8))
            oanpool = ctx.enter_context(tc.tile_pool(name="oan", bufs=2))
            bcpool = ctx.enter_context(tc.tile_pool(name="bc", bufs=2))
            rcpool = ctx.enter_context(tc.tile_pool(name="rc", bufs=4))
            ocpool = ctx.enter_context(tc.tile_pool(name="oc", bufs=4))
            # PSUM: 2*[128,1024] (4 banks) + 2*[65,512] (2) + 2*[128,512] (2)
            attp = ctx.enter_context(tc.tile_pool(name="attp", bufs=2, space="PSUM"))
            oaugp = ctx.enter_context(tc.tile_pool(name="oaugp", bufs=2, space="PSUM"))
            projp = ctx.enter_context(tc.tile_pool(name="projp", bufs=2, space="PSUM"))

            # ---- SBUF homes ----
            wq_sb = consts.tile([128, NDC, FS], BF16)
            wk_sb = consts.tile([128, NDC, FS], BF16)
            wv_sb = consts.tile([128, NDC, NJ * 65], BF16)
            wo_sb = consts.tile([128, 2, D], BF16)
            bq_sb = consts.tile([128, 2], F32)
            bk_sb = consts.tile([128, 2], F32)
            mask_sb = consts.tile([128, NJ * 65], F32)
            wu = consts.tile([128, 64], BF16)

            qt_sb = acts.tile([128, 2, S], BF16)
            kt_sb = acts.tile([128, 2, S], BF16)
            v_sb = acts.tile([128, NKC, NJ * 65], BF16)
            ot_sb = acts.tile([128, 2, S], BF16)

            xq_t = [xpool.tile([128, S], BF16, tag="x", name=f"xq{c}")
                    for c in range(NDC)]
            xk_t = [xpool.tile([128, S], BF16, tag="x", name=f"xk{c}")
                    for c in range(NDC)]
            xv_t = [xpool.tile([128, S], BF16, tag="x", name=f"xv{c}")
                    for c in range(NDC)]

            # ---- DMA issue, priority order, alternating sync/gpsimd ----
            def dma(i, out, in_):
                (nc.gpsimd if i % 2 else nc.sync).dma_start(out=out, in_=in_)

            dma(0, wq_sb[:], wq_d[:].rearrange("(c p) m -> p c m", p=128))
            dma(1, bq_sb[:], bq_d[:])
            for c in range(NDC):
                dma(c, xq_t[c][:, 0:512], xq_d[ts(c, 128), 0:512])
            dma(0, wk_sb[:], wk_d[:].rearrange("(c p) m -> p c m", p=128))
            dma(1, bk_sb[:], bk_d[:])
            for c in range(NDC):
                dma(c + 1, xk_t[c][:, 0:256], xk_d[ts(c, 128), 0:256])
            for c in range(NDC):
                dma(c + 1, xk_t[c][:, 256:512], xk_d[ts(c, 128), 256:512])
            dma(1, wv_sb[:], wv_d[:].rearrange("(c p) m -> p c m", p=128))
            dma(0, mask_sb[:], mask_d[:])
            for c in range(NDC):
                dma(c, xv_t[c][:, 0:256], xv_d[ts(c, 128), 0:256])
            for c in range(NDC):
                dma(c, xv_t[c][:, 256:1024], xv_d[ts(c, 128), 256:1024])
            for c in range(NDC):
                dma(c + 1, xk_t[c][:, 512:1024], xk_d[ts(c, 128), 512:1024])
            for c in range(NDC):
                dma(c + 1, xk_t[c][:, 1024:S], xk_d[ts(c, 128), 1024:S])
            for c in range(NDC):
                dma(c, xv_t[c][:, 1024:S], xv_d[ts(c, 128), 1024:S])
            for c in range(NDC):
                dma(c + 1, xq_t[c][:, 512:1024], xq_d[ts(c, 128), 512:1024])
            dma(0, wo_sb[:], wo_d[:].rearrange("(c p) m -> p c m", p=128))
            for c in range(NDC):
                dma(c + 1, xq_t[c][:, 1024:S], xq_d[ts(c, 128), 1024:S])

            # ---- PE warm-up (p-state ramp) while DMAs land ----
            nc.vector.memset(wu[:], 0.25)
            wups = projp.tile([128, 512], F32, tag="proj", name="wups")
            for _ in range(32):
                nc.tensor.matmul(wups[:64, 0:64], wu[:, 0:64], wu[:, 0:64],
                                 start=True, stop=True)

            # ---- projection chunk emitters ----
            def kq_chunk(w_sb, x_t, b_sb, dst, ft, c0, w, nm):
                ps = projp.tile([128, 512], F32, tag="proj", name=nm)
                for dc in range(NDC):
                    nc.tensor.matmul(ps[:, 0:w], w_sb[:, dc, ts(ft, 128)],
                                     x_t[dc][:, c0:c0 + w],
                                     start=(dc == 0), stop=(dc == NDC - 1))
                nc.vector.tensor_scalar_add(dst[:, ft, c0:c0 + w], ps[:, 0:w],
                                            b_sb[:, ft:ft + 1])

            def v_chunk(kc, nm):
                ps = projp.tile([128, 512], F32, tag="proj", name=nm)
                for dc in range(NDC):
                    nc.tensor.matmul(ps[:, 0:NJ * 65], xv_t[dc][:, ts(kc, 128)],
                                     wv_sb[:, dc, :],
                                     start=(dc == 0), stop=(dc == NDC - 1))
                nc.vector.tensor_add(v_sb[:, kc, :], ps[:, 0:NJ * 65], mask_sb[:])

            def wo_chain(qg, nm, copy_eng=None, tail=False, half=None):
                # half=0: emit only the ft0 matmul (chain stays open in its
                # projp slot); half=1: finish with ft1 + copy + out-DMA
                if half in (None, 0):
                    ps = projp.tile([128, 512], F32, tag="proj", name=nm)
                    wo_open[qg] = ps
                else:
                    ps = wo_open.pop(qg)
                for ft in ((0, 1) if half is None else (half,)):
                    nc.tensor.matmul(ps[:], ot_sb[:, ft, ts(qg, 128)],
                                     wo_sb[:, ft, :],
                                     start=(ft == 0), stop=(ft == 1))
                if half == 0:
                    return
                oc = ocpool.tile([128, D], BF16, tag="oc", name=f"oc{qg}")
                eng = copy_eng if copy_eng is not None else nc.vector
                if eng is nc.scalar:
                    eng.copy(out=oc[:], in_=ps[:])
                else:
                    eng.tensor_copy(oc[:], ps[:])
                # issue out-DMA on sync (tail) / gpsimd (mid-run)
                deng = nc.sync if tail else nc.gpsimd
                deng.dma_start(out=out_d[ts(qg, 128), :], in_=oc[:])

            wo_open = {}

            # ---- startup projections (before first EXP) ----
            kq_chunk(wq_sb, xq_t, bq_sb, qt_sb, 0, 0, 512, "q000")
            kq_chunk(wk_sb, xk_t, bk_sb, kt_sb, 0, 0, 256, "k00")
            kq_chunk(wk_sb, xk_t, bk_sb, kt_sb, 0, 256, 256, "k01")

            # ---- filler schedule ----
            fillers = [[] for _ in range(NIT + 1)]

            def kf(ft, c):
                return lambda: kq_chunk(wk_sb, xk_t, bk_sb, kt_sb, ft,
                                        256 * c, 256, f"k{ft}{c}")

            def qf(ft, qc):
                return lambda: kq_chunk(wq_sb, xq_t, bq_sb, qt_sb, ft,
                                        QW * qc, QW, f"q{ft}{qc}")

            def qf256(ft, qc, h):
                return lambda: kq_chunk(wq_sb, xq_t, bq_sb, qt_sb, ft,
                                        QW * qc + 256 * h, 256,
                                        f"q{ft}{qc}{h}")

            # sweep 0: V0,V1 first, then K-ft0 c2..7 JIT (chunk c needed at
            # it 2c) and the remaining V chunks; late: K-ft1 c0,c1 and
            # Q-ft1-qc0 (needed at it 16)
            for c in range(2, 8):
                fillers[2 * c - 3].append(kf(0, c))
            vsched = [0, 1, 3, 4, 5, 6, 7, 7, 8, 8, 9, 9, 10, 10, 11, 12]
            for j in range(NKC):
                fillers[vsched[j]].append(lambda j=j: v_chunk(j, f"v{j}"))
            fillers[8].append(qf(1, 0))
            fillers[13].append(kf(1, 0))
            fillers[14].append(kf(1, 1))
            # sweep 1: K-ft1 c2..7 JIT (needed at it 16+2c)
            for c in range(2, 8):
                fillers[2 * c + 12].append(kf(1, c))
            # Q chunks (two atomic 256-wide chunks each): (ft, qc) needed
            # at sweep 2*qc+ft start (it 32qc+16ft)
            for i, (ft, qc) in enumerate([(0, 1), (1, 1), (0, 2), (1, 2),
                                          (0, 3), (1, 3)]):
                base = [17, 28, 44, 58, 74, 90][i]
                fillers[base].append(qf256(ft, qc, 0))
                fillers[base + 1].append(qf256(ft, qc, 1))
            # keep-warm PE fillers in otherwise-empty late iterations
            # (PE micro-idle drops the clock and slows every engine)
            def warm():
                ps = projp.tile([128, 512], F32, tag="proj", name="warm")
                nc.tensor.matmul(ps[:64, :], wu[:, 0:64], kt_sb[:, 0, 0:512],
                                 start=True, stop=True)

            for it in range(96, 112):
                if it not in {102, 103, 105, 106, 108, 109, 111}:
                    fillers[it].append(warm)
            # wo chains: qc_i blocks ready ~5us after norm(s=2i+1) at
            # it 32i+33; spread across the following sweeps
            for i in range(3):
                for b in range(4):
                    fillers[32 * i + 39 + 3 * b].append(
                        lambda qg=4 * i + b: wo_chain(qg, f"wo{qg}"))

            # ---- normalization (per sweep: both heads of the pair) ----
            def norm(s, oaugs, last=False):
                qc, jp = s // 2, s % 2
                oan = oanpool.tile([65, 1024], F32, tag="oan", name=f"oan{s}")
                if last:
                    nc.scalar.copy(out=oan[64:65, 0:512], in_=oaugs[0][64:65, :])
                    nc.vector.tensor_copy(oan[64:65, 512:1024],
                                          oaugs[1][64:65, :])
                else:
                    nc.vector.tensor_copy(oan[:, 0:512], oaugs[0][:])
                    nc.vector.tensor_copy(oan[:, 512:1024], oaugs[1][:])
                dn = rcpool.tile([64, 16], F32, tag="dn", name=f"dn{s}")
                nc.sync.dma_start(out=dn[:], in_=oan[64:65, :])
                if last:
                    nc.vector.tensor_copy(oan[0:64, 0:512], oaugs[0][0:64, :])
                    nc.vector.tensor_copy(oan[0:64, 512:1024], oaugs[1][0:64, :])
                nc.vector.reciprocal(dn[:], dn[:])
                rc = rcpool.tile([1, 1024], F32, tag="rc", name=f"rc{s}")
                nc.sync.dma_start(out=rc[:], in_=dn[:])
                bc = bcpool.tile([64, 1024], F32, tag="bc", name=f"bcn{s}")
                if last:
                    nc.gpsimd.partition_broadcast(bc[:, 0:512], rc[:, 0:512],
                                                  channels=64)
                    nc.gpsimd.partition_broadcast(bc[:, 512:1024],
                                                  rc[:, 512:1024], channels=64)
                else:
                    nc.gpsimd.partition_broadcast(bc[:], rc[:], channels=64)
                if not last:
                    for hi in range(2):
                        nc.vector.tensor_mul(
                            ot_sb[hi * 64:hi * 64 + 64, jp,
                                  qc * QW:(qc + 1) * QW],
                            oan[0:64, hi * 512:hi * 512 + 512],
                            bc[:, hi * 512:hi * 512 + 512])
                else:
                    last_norm.append((oan, bc))

            # ---- main pipeline ----
            # AV(s, kc) emitted at it 16s + kc + lag; lags shrink with kc
            # (never faster than +1 per kc -> psum accumulation stays ordered)
            def av_lag(s, kc):
                if s == 0:
                    return max(6 - kc // 2, 3)
                return 4 if kc < 4 else 3

            av_emit = [[] for _ in range(NIT + 7)]
            for s in range(NSW):
                for kc in range(NKC):
                    av_emit[16 * s + kc + av_lag(s, kc)].append((s, kc))

            es = {}        # (s, kc) -> e tile
            oaug_t = {}    # s -> [oaug_h0, oaug_h1]
            last_norm = []

            def emit_av(s, kc):
                qc, jp = s // 2, s % 2
                if kc == 0:
                    oaug_t[s] = [oaugp.tile([65, QW], F32, tag="oaug",
                                            name=f"oaug{s}h{hi}")
                                 for hi in range(2)]
                e = es.pop((s, kc))
                for hi in range(2):
                    nc.tensor.matmul(
                        oaug_t[s][hi][:],
                        v_sb[:, kc, (2 * jp + hi) * 65:(2 * jp + hi) * 65 + 65],
                        e[:, hi * 512:hi * 512 + 512],
                        start=(kc == 0), stop=(kc == NKC - 1))
                if kc == NKC - 1:
                    norm(s, oaug_t[s], last=(s == NSW - 1))

            for it in range(NIT):
                s, kc = it // NKC, it % NKC
                qc, jp = s // 2, s % 2
                # row-tiled QK pair: heads 2jp (rows 0:64) and 2jp+1 (64:128)
                att = attp.tile([128, 1024], F32, tag="att", name=f"att{it}")
                for hi in range(2):
                    p0 = hi * 64
                    nc.tensor.matmul(
                        att[:, hi * 512:hi * 512 + 512],
                        kt_sb[p0:p0 + 64, jp, ts(kc, 128)],
                        qt_sb[p0:p0 + 64, jp, qc * QW:(qc + 1) * QW],
                        start=True, stop=True)
                e = epool.tile([128, 1024], BF16, tag="e", name=f"e{it}")
                nc.scalar.activation(out=e[:], in_=att[:], func=AF.Exp)
                es[(s, kc)] = e
                for (s2, kc2) in av_emit[it]:
                    emit_av(s2, kc2)
                for f in fillers[it]:
                    f()

            for itx in range(NIT, NIT + 7):
                for (s2, kc2) in av_emit[itx]:
                    emit_av(s2, kc2)

            # ---- tail: last qc norm-muls pipelined with the last 4 wo ----
            wut = projp.tile([128, 512], F32, tag="proj", name="wut")
            for _ in range(30):
                nc.tensor.matmul(wut[:64, 0:256], wu[:, 0:64],
                                 kt_sb[:, 0, 0:256], start=True, stop=True)
            oan, bc = last_norm[0]
            qc3 = (NQC - 1) * QW
            # per-256-col: both heads' muls, then the 2 wo blocks they unlock
            for m in range(2):
                for hi in range(2):
                    c0 = hi * 512 + m * 256
                    nc.vector.tensor_mul(
                        ot_sb[hi * 64:hi * 64 + 64, 1,
                              qc3 + m * 256:qc3 + m * 256 + 256],
                        oan[0:64, c0:c0 + 256], bc[:, c0:c0 + 256])
                for b in (12 + 2 * m, 13 + 2 * m):
                    wo_chain(b, f"wot{b}",
                             copy_eng=(nc.scalar if b % 2 == 0 else nc.vector),
                             tail=True)

    nc.finalize()
    return nc


def prepare_core_inputs(q, k, v, wq, wk, wv, wo, bq, bk, bv, bo):
    """Numpy host-side sharding/layout prep. Returns (in_maps, bo_eff)."""
    bf16 = np.float16
    mask = np.zeros((128, NJ * 65), np.float32)
    for j in range(NJ):
        mask[:, j * 65 + 64] = 1.0
    in_maps = []
    for c in range(8):
        b, g = c // 2, c % 2
        fs = slice(g * FS, (g + 1) * FS)
        wv_aug = np.zeros((D, NJ * 65), np.float32)
        wv_g = wv[fs, :]  # [256, 512]
        for j in range(NJ):
            wv_aug[:, j * 65: j * 65 + 64] = wv_g[j * 64:(j + 1) * 64, :].T
        in_maps.append({
            "xq": np.ascontiguousarray(q[b].T).astype(bf16),
            "xk": np.ascontiguousarray(k[b].T).astype(bf16),
            "xv": np.ascontiguousarray(v[b].T).astype(bf16),
            "wq": np.ascontiguousarray((wq[fs, :] / 8.0).T).astype(bf16),
            "wk": np.ascontiguousarray(wk[fs, :].T).astype(bf16),
            "wv": np.ascontiguousarray(wv_aug).astype(bf16),
            "wo": np.ascontiguousarray(wo[:, fs].T).astype(bf16),
            "bq": np.ascontiguousarray((bq[fs] / 8.0).reshape(2, 128).T, np.float32),
            "bk": np.ascontiguousarray(bk[fs].reshape(2, 128).T, np.float32),
            "mask": mask,
        })
    bo_eff = (bo.astype(np.float32)
              + bv.astype(np.float32) @ wo.astype(np.float32).T)
    return in_maps, bo_eff


def kernel(q, k, v, wq, wk, wv, wo, bq, bk, bv, bo):
    q, k, v = (np.asarray(x, np.float32) for x in (q, k, v))
    wq, wk, wv, wo = (np.asarray(x, np.float32) for x in (wq, wk, wv, wo))
    bq, bk, bv, bo = (np.asarray(x, np.float32) for x in (bq, bk, bv, bo))

    if "nc" not in _cache:
        _cache["nc"] = build_nc()
    nc = _cache["nc"]

    in_maps, bo_eff = prepare_core_inputs(q, k, v, wq, wk, wv, wo, bq, bk, bv, bo)
    res = run_bass_kernel_spmd(nc, in_maps, list(range(8)))
    _cache["last_results"] = res

    out = np.empty((B, S, D), np.float32)
    for b in range(B):
        out[b] = (res.results[2 * b]["out"].astype(np.float32)
                  + res.results[2 * b + 1]["out"].astype(np.float32) + bo_eff)
    return out


# revision 33
# speedup vs baseline: 1.1756x; 1.1756x over previous
"""Multi-head attention (B=4, S=2048, D=512, H=8) on 8 trn2 NeuronCores.

Sharding: core c -> batch b = c//2, feature-slice g = c%2 (256 features =
4 heads).  Each core computes Q/K/V projections for its 4 heads, a
flash-style streaming softmax-attention (no S x S materialization in HBM),
and a partial output projection through its 256-column slice of wo.  The
host sums the two partials per batch and adds the (bo + bv @ wo.T) constant.

v4: single software-pipelined loop of 128 (qc512, jp, kc) iterations.
Each iteration runs the head PAIR of jp: two row-tiled QK matmuls (heads
at PE rows 0:64 / 64:128 overlap in the array), ONE [128,1024] EXP
covering both heads, and two [65,512] AV accumulations.  K/Q/V projection
chunks and wo chains are emitted as PE filler at scheduled iterations;
DMAs issue in priority order on the sync + gpsimd sequencers.  PSUM:
2x[128,1024] QK double-buffer + 2x[65,512] AV + 2x[128,512] proj = 8 banks.

All activations stay "transposed" ([feature, seq]) on device:
  QT = (wq/8 @ x_q.T + bq/8), KT = wk @ x_k.T + bk          [256, 2048]
  attT_h = K_h @ Q_h.T  (k on partitions, q free)            [2048, 2048]
  E = exp(attT)  (no max subtraction: logits ~ N(0,1))
  [outT; denom] = [V_h | 1].T @ E  (ones column -> denominators)
  outT_norm = outT * (1/denom broadcast)                     [256, 2048]
  out_partial = outT_norm.T @ (wo slice)                     [2048, 512]

Numerics: QK^T in fp32 psum; E / V / weights / partial outputs in fp16.
"""

import os
import sys

for _p in ("/opt/trn_rl_repo", "/root/.axon_site/_ro/trn_rl_repo"):
    if os.path.isdir(_p) and _p not in sys.path:
        sys.path.append(_p)

import numpy as np

import concourse.bass as bass
import concourse.bacc as bacc
import concourse.tile as tile
import concourse.mybir as mybir
from concourse.bass import ts
from concourse.bass_utils import run_bass_kernel_spmd

F32 = mybir.dt.float32
BF16 = mybir.dt.float16
AF = mybir.ActivationFunctionType

B, S, D = 4, 2048, 512
NH, DK = 8, 64
FS = 256           # features per core (4 heads)
NJ = 4             # local heads
QW = 512           # q-chunk per sweep
NQC = S // QW      # 4
NKC = S // 128     # 16 k-chunks
NDC = D // 128     # 4 contraction chunks for projections
NSW = NQC * 2      # 8 sweeps (qc, jp)
NIT = NSW * NKC    # 128 pipeline iterations

_cache = {}


def build_nc():
    nc = bacc.Bacc("TRN2", target_bir_lowering=False, debug=False)

    xq_d = nc.dram_tensor("xq", [D, S], BF16, kind="ExternalInput")
    xk_d = nc.dram_tensor("xk", [D, S], BF16, kind="ExternalInput")
    xv_d = nc.dram_tensor("xv", [D, S], BF16, kind="ExternalInput")
    wq_d = nc.dram_tensor("wq", [D, FS], BF16, kind="ExternalInput")
    wk_d = nc.dram_tensor("wk", [D, FS], BF16, kind="ExternalInput")
    wv_d = nc.dram_tensor("wv", [D, NJ * 65], BF16, kind="ExternalInput")
    wo_d = nc.dram_tensor("wo", [FS, D], BF16, kind="ExternalInput")
    bq_d = nc.dram_tensor("bq", [128, 2], F32, kind="ExternalInput")
    bk_d = nc.dram_tensor("bk", [128, 2], F32, kind="ExternalInput")
    mask_d = nc.dram_tensor("mask", [128, NJ * 65], F32, kind="ExternalInput")
    out_d = nc.dram_tensor("out", [S, D], BF16, kind="ExternalOutput")

    with tile.TileContext(nc) as tc:
        from contextlib import ExitStack
        with ExitStack() as ctx:
            consts = ctx.enter_context(tc.tile_pool(name="consts", bufs=1))
            xpool = ctx.enter_context(tc.tile_pool(name="x", bufs=12))
            acts = ctx.enter_context(tc.tile_pool(name="acts", bufs=1))
            epool = ctx.enter_context(tc.tile_pool(name="expatt", bufs=8))
            oanpool = ctx.enter_context(tc.tile_pool(name="oan", bufs=2))
            bcpool = ctx.enter_context(tc.tile_pool(name="bc", bufs=2))
            rcpool = ctx.enter_context(tc.tile_pool(name="rc", bufs=4))
            ocpool = ctx.enter_context(tc.tile_pool(name="oc", bufs=4))
            # PSUM: 2*[128,1024] (4 banks) + 2*[65,512] (2) + 2*[128,512] (2)
            attp = ctx.enter_context(tc.tile_pool(name="attp", bufs=2, space="PSUM"))
            oaugp = ctx.enter_context(tc.tile_pool(name="oaugp", bufs=2, space="PSUM"))
            projp = ctx.enter_context(tc.tile_pool(name="projp", bufs=2, space="PSUM"))

            # ---- SBUF homes ----
            wq_sb = consts.tile([128, NDC, FS], BF16)
            wk_sb = consts.tile([128, NDC, FS], BF16)
            wv_sb = consts.tile([128, NDC, NJ * 65], BF16)
            wo_sb = consts.tile([128, 2, D], BF16)
            bq_sb = consts.tile([128, 2], F32)
            bk_sb = consts.tile([128, 2], F32)
            mask_sb = consts.tile([128, NJ * 65], F32)
            wu = consts.tile([128, 64], BF16)

            qt_sb = acts.tile([128, 2, S], BF16)
            kt_sb = acts.tile([128, 2, S], BF16)
            v_sb = acts.tile([128, NKC, NJ * 65], BF16)
            ot_sb = acts.tile([128, 2, S], BF16)

            xq_t = [xpool.tile([128, S], BF16, tag="x", name=f"xq{c}")
                    for c in range(NDC)]
            xk_t = [xpool.tile([128, S], BF16, tag="x", name=f"xk{c}")
                    for c in range(NDC)]
            xv_t = [xpool.tile([128, S], BF16, tag="x", name=f"xv{c}")
                    for c in range(NDC)]

            # ---- DMA issue, priority order, alternating sync/gpsimd ----
            def dma(i, out, in_):
                (nc.gpsimd if i % 2 else nc.sync).dma_start(out=out, in_=in_)

            dma(0, wq_sb[:], wq_d[:].rearrange("(c p) m -> p c m", p=128))
            dma(1, bq_sb[:], bq_d[:])
            for c in range(NDC):
                dma(c, xq_t[c][:, 0:512], xq_d[ts(c, 128), 0:512])
            dma(0, wk_sb[:], wk_d[:].rearrange("(c p) m -> p c m", p=128))
            dma(1, bk_sb[:], bk_d[:])
            for c in range(NDC):
                dma(c + 1, xk_t[c][:, 0:256], xk_d[ts(c, 128), 0:256])
            dma(1, wv_sb[:], wv_d[:].rearrange("(c p) m -> p c m", p=128))
            dma(0, mask_sb[:], mask_d[:])
            for c in range(NDC):
                dma(c, xv_t[c][:, 0:256], xv_d[ts(c, 128), 0:256])
            for c in range(NDC):
                dma(c + 1, xk_t[c][:, 256:512], xk_d[ts(c, 128), 256:512])
            for c in range(NDC):
                dma(c, xv_t[c][:, 256:1024], xv_d[ts(c, 128), 256:1024])
            for c in range(NDC):
                dma(c + 1, xk_t[c][:, 512:1024], xk_d[ts(c, 128), 512:1024])
            for c in range(NDC):
                dma(c + 1, xk_t[c][:, 1024:S], xk_d[ts(c, 128), 1024:S])
            for c in range(NDC):
                dma(c, xv_t[c][:, 1024:S], xv_d[ts(c, 128), 1024:S])
            for c in range(NDC):
                dma(c + 1, xq_t[c][:, 512:1024], xq_d[ts(c, 128), 512:1024])
            dma(0, wo_sb[:], wo_d[:].rearrange("(c p) m -> p c m", p=128))
            for c in range(NDC):
                dma(c + 1, xq_t[c][:, 1024:S], xq_d[ts(c, 128), 1024:S])

            # ---- PE warm-up (p-state ramp) while DMAs land ----
            nc.vector.memset(wu[:], 0.25)
            wups = projp.tile([128, 512], F32, tag="proj", name="wups")
            for _ in range(32):
                nc.tensor.matmul(wups[:64, 0:64], wu[:, 0:64], wu[:, 0:64],
                                 start=True, stop=True)

            # ---- projection chunk emitters ----
            def kq_chunk(w_sb, x_t, b_sb, dst, ft, c0, w, nm):
                ps = projp.tile([128, 512], F32, tag="proj", name=nm)
                for dc in range(NDC):
                    nc.tensor.matmul(ps[:, 0:w], w_sb[:, dc, ts(ft, 128)],
                                     x_t[dc][:, c0:c0 + w],
                                     start=(dc == 0), stop=(dc == NDC - 1))
                nc.vector.tensor_scalar_add(dst[:, ft, c0:c0 + w], ps[:, 0:w],
                                            b_sb[:, ft:ft + 1])

            def v_chunk(kc, nm):
                ps = projp.tile([128, 512], F32, tag="proj", name=nm)
                for dc in range(NDC):
                    nc.tensor.matmul(ps[:, 0:NJ * 65], xv_t[dc][:, ts(kc, 128)],
                                     wv_sb[:, dc, :],
                                     start=(dc == 0), stop=(dc == NDC - 1))
                nc.vector.tensor_add(v_sb[:, kc, :], ps[:, 0:NJ * 65], mask_sb[:])

            def wo_chain(qg, nm, copy_eng=None, tail=False, half=None):
                # half=0: emit only the ft0 matmul (chain stays open in its
                # projp slot); half=1: finish with ft1 + copy + out-DMA
                if half in (None, 0):
                    ps = projp.tile([128, 512], F32, tag="proj", name=nm)
                    wo_open[qg] = ps
                else:
                    ps = wo_open.pop(qg)
                for ft in ((0, 1) if half is None else (half,)):
                    nc.tensor.matmul(ps[:], ot_sb[:, ft, ts(qg, 128)],
                                     wo_sb[:, ft, :],
                                     start=(ft == 0), stop=(ft == 1))
                if half == 0:
                    return
                oc = ocpool.tile([128, D], BF16, tag="oc", name=f"oc{qg}")
                eng = copy_eng if copy_eng is not None else nc.vector
                if eng is nc.scalar:
                    eng.copy(out=oc[:], in_=ps[:])
                else:
                    eng.tensor_copy(oc[:], ps[:])
                # issue out-DMA on sync (tail) / gpsimd (mid-run)
                deng = nc.sync if tail else nc.gpsimd
                deng.dma_start(out=out_d[ts(qg, 128), :], in_=oc[:])

            wo_open = {}

            # ---- startup projections (before first EXP) ----
            kq_chunk(wq_sb, xq_t, bq_sb, qt_sb, 0, 0, 512, "q000")
            kq_chunk(wk_sb, xk_t, bk_sb, kt_sb, 0, 0, 256, "k00")
            kq_chunk(wk_sb, xk_t, bk_sb, kt_sb, 0, 256, 256, "k01")

            # ---- filler schedule ----
            fillers = [[] for _ in range(NIT + 1)]

            def kf(ft, c):
                return lambda: kq_chunk(wk_sb, xk_t, bk_sb, kt_sb, ft,
                                        256 * c, 256, f"k{ft}{c}")

            def qf(ft, qc):
                return lambda: kq_chunk(wq_sb, xq_t, bq_sb, qt_sb, ft,
                                        QW * qc, QW, f"q{ft}{qc}")

            def qf256(ft, qc, h):
                return lambda: kq_chunk(wq_sb, xq_t, bq_sb, qt_sb, ft,
                                        QW * qc + 256 * h, 256,
                                        f"q{ft}{qc}{h}")

            # sweep 0: V0,V1 first, then K-ft0 c2..7 JIT (chunk c needed at
            # it 2c) and the remaining V chunks; late: K-ft1 c0,c1 and
            # Q-ft1-qc0 (needed at it 16)
            for c in range(2, 8):
                fillers[2 * c - 3].append(kf(0, c))
            vsched = [0, 1, 3, 4, 5, 6, 7, 7, 8, 8, 9, 9, 10, 10, 11, 12]
            for j in range(NKC):
                fillers[vsched[j]].append(lambda j=j: v_chunk(j, f"v{j}"))
            fillers[8].append(qf(1, 0))
            fillers[13].append(kf(1, 0))
            fillers[14].append(kf(1, 1))
            # sweep 1: K-ft1 c2..7 JIT (needed at it 16+2c)
            for c in range(2, 8):
                fillers[2 * c + 12].append(kf(1, c))
            # Q chunks (two atomic 256-wide chunks each): (ft, qc) needed
            # at sweep 2*qc+ft start (it 32qc+16ft)
            for i, (ft, qc) in enumerate([(0, 1), (1, 1), (0, 2), (1, 2),
                                          (0, 3), (1, 3)]):
                base = [17, 28, 44, 58, 74, 90][i]
                fillers[base].append(qf256(ft, qc, 0))
                fillers[base + 1].append(qf256(ft, qc, 1))
            # keep-warm PE fillers in otherwise-empty late iterations
            # (PE micro-idle drops the clock and slows every engine)
            def warm():
                ps = projp.tile([128, 512], F32, tag="proj", name="warm")
                nc.tensor.matmul(ps[:64, :], wu[:, 0:64], kt_sb[:, 0, 0:512],
                                 start=True, stop=True)

            for it in range(96, NIT):
                if it not in {102, 103, 105, 106, 108, 109, 111, 112}:
                    fillers[it].append(warm)
            # wo chains: qc_i blocks ready ~5us after norm(s=2i+1) at
            # it 32i+33; spread across the following sweeps
            for i in range(3):
                for b in range(4):
                    fillers[32 * i + 39 + 3 * b].append(
                        lambda qg=4 * i + b: wo_chain(qg, f"wo{qg}"))

            # ---- normalization (per sweep: both heads of the pair) ----
            def norm(s, oaugs, last=False):
                qc, jp = s // 2, s % 2
                oan = oanpool.tile([65, 1024], F32, tag="oan", name=f"oan{s}")
                if last:
                    nc.scalar.copy(out=oan[64:65, 0:512], in_=oaugs[0][64:65, :])
                    nc.vector.tensor_copy(oan[64:65, 512:1024],
                                          oaugs[1][64:65, :])
                else:
                    nc.vector.tensor_copy(oan[:, 0:512], oaugs[0][:])
                    nc.vector.tensor_copy(oan[:, 512:1024], oaugs[1][:])
                dn = rcpool.tile([64, 16], F32, tag="dn", name=f"dn{s}")
                nc.sync.dma_start(out=dn[:], in_=oan[64:65, :])
                if last:
                    nc.vector.tensor_copy(oan[0:64, 0:512], oaugs[0][0:64, :])
                    nc.vector.tensor_copy(oan[0:64, 512:1024], oaugs[1][0:64, :])
                nc.vector.reciprocal(dn[:], dn[:])
                rc = rcpool.tile([1, 1024], F32, tag="rc", name=f"rc{s}")
                nc.sync.dma_start(out=rc[:], in_=dn[:])
                bc = bcpool.tile([64, 1024], F32, tag="bc", name=f"bcn{s}")
                if last:
                    nc.gpsimd.partition_broadcast(bc[:, 0:512], rc[:, 0:512],
                                                  channels=64)
                    nc.gpsimd.partition_broadcast(bc[:, 512:1024],
                                                  rc[:, 512:1024], channels=64)
                else:
                    nc.gpsimd.partition_broadcast(bc[:], rc[:], channels=64)
                if not last:
                    for hi in range(2):
                        nc.vector.tensor_mul(
                            ot_sb[hi * 64:hi * 64 + 64, jp,
                                  qc * QW:(qc + 1) * QW],
                            oan[0:64, hi * 512:hi * 512 + 512],
                            bc[:, hi * 512:hi * 512 + 512])
                else:
                    last_norm.append((oan, bc))

            # ---- main pipeline ----
            # AV(s, kc) emitted at it 16s + kc + lag; lags shrink with kc
            # (never faster than +1 per kc -> psum accumulation stays ordered)
            def av_lag(s, kc):
                if s == 0:
                    return max(6 - kc // 2, 2)
                return 4 if kc < 2 else (3 if kc < 4 else 2)

            av_emit = [[] for _ in range(NIT + 7)]
            for s in range(NSW):
                for kc in range(NKC):
                    av_emit[16 * s + kc + av_lag(s, kc)].append((s, kc))

            es = {}        # (s, kc) -> e tile
            oaug_t = {}    # s -> [oaug_h0, oaug_h1]
            last_norm = []

            def emit_av(s, kc):
                qc, jp = s // 2, s % 2
                if kc == 0:
                    oaug_t[s] = [oaugp.tile([65, QW], F32, tag="oaug",
                                            name=f"oaug{s}h{hi}")
                                 for hi in range(2)]
                e = es.pop((s, kc))
                for hi in range(2):
                    nc.tensor.matmul(
                        oaug_t[s][hi][:],
                        v_sb[:, kc, (2 * jp + hi) * 65:(2 * jp + hi) * 65 + 65],
                        e[:, hi * 512:hi * 512 + 512],
                        start=(kc == 0), stop=(kc == NKC - 1))
                if kc == NKC - 1:
                    norm(s, oaug_t[s], last=(s == NSW - 1))

            for it in range(NIT):
                s, kc = it // NKC, it % NKC
                qc, jp = s // 2, s % 2
                # row-tiled QK pair: heads 2jp (rows 0:64) and 2jp+1 (64:128)
                att = attp.tile([128, 1024], F32, tag="att", name=f"att{it}")
                for hi in range(2):
                    p0 = hi * 64
                    nc.tensor.matmul(
                        att[:, hi * 512:hi * 512 + 512],
                        kt_sb[p0:p0 + 64, jp, ts(kc, 128)],
                        qt_sb[p0:p0 + 64, jp, qc * QW:(qc + 1) * QW],
                        start=True, stop=True)
                e = epool.tile([128, 1024], BF16, tag="e", name=f"e{it}")
                nc.scalar.activation(out=e[:], in_=att[:], func=AF.Exp)
                es[(s, kc)] = e
                for (s2, kc2) in av_emit[it]:
                    emit_av(s2, kc2)
                for f in fillers[it]:
                    f()

            for itx in range(NIT, NIT + 7):
                for (s2, kc2) in av_emit[itx]:
                    emit_av(s2, kc2)

            # ---- tail: last qc norm-muls pipelined with the last 4 wo ----
            wut = projp.tile([128, 512], F32, tag="proj", name="wut")
            for _ in range(30):
                nc.tensor.matmul(wut[:64, 0:256], wu[:, 0:64],
                                 kt_sb[:, 0, 0:256], start=True, stop=True)
            oan, bc = last_norm[0]
            qc3 = (NQC - 1) * QW
            # per-256-col: both heads' muls, then the 2 wo blocks they unlock
            for m in range(2):
                for hi in range(2):
                    c0 = hi * 512 + m * 256
                    nc.vector.tensor_mul(
                        ot_sb[hi * 64:hi * 64 + 64, 1,
                              qc3 + m * 256:qc3 + m * 256 + 256],
                        oan[0:64, c0:c0 + 256], bc[:, c0:c0 + 256])
                for b in (12 + 2 * m, 13 + 2 * m):
                    wo_chain(b, f"wot{b}",
                             copy_eng=(nc.scalar if b % 2 == 0 else nc.vector),
                             tail=True)

    nc.finalize()
    return nc


def prepare_core_inputs(q, k, v, wq, wk, wv, wo, bq, bk, bv, bo):
    """Numpy host-side sharding/layout prep. Returns (in_maps, bo_eff)."""
    bf16 = np.float16
    mask = np.zeros((128, NJ * 65), np.float32)
    for j in range(NJ):
        mask[:, j * 65 + 64] = 1.0
    in_maps = []
    for c in range(8):
        b, g = c // 2, c % 2
        fs = slice(g * FS, (g + 1) * FS)
        wv_aug = np.zeros((D, NJ * 65), np.float32)
        wv_g = wv[fs, :]  # [256, 512]
        for j in range(NJ):
            wv_aug[:, j * 65: j * 65 + 64] = wv_g[j * 64:(j + 1) * 64, :].T
        in_maps.append({
            "xq": np.ascontiguousarray(q[b].T).astype(bf16),
            "xk": np.ascontiguousarray(k[b].T).astype(bf16),
            "xv": np.ascontiguousarray(v[b].T).astype(bf16),
            "wq": np.ascontiguousarray((wq[fs, :] / 8.0).T).astype(bf16),
            "wk": np.ascontiguousarray(wk[fs, :].T).astype(bf16),
            "wv": np.ascontiguousarray(wv_aug).astype(bf16),
            "wo": np.ascontiguousarray(wo[:, fs].T).astype(bf16),
            "bq": np.ascontiguousarray((bq[fs] / 8.0).reshape(2, 128).T, np.float32),
            "bk": np.ascontiguousarray(bk[fs].reshape(2, 128).T, np.float32),
            "mask": mask,
        })
    bo_eff = (bo.astype(np.float32)
              + bv.astype(np.float32) @ wo.astype(np.float32).T)
    return in_maps, bo_eff


def kernel(q, k, v, wq, wk, wv, wo, bq, bk, bv, bo):
    q, k, v = (np.asarray(x, np.float32) for x in (q, k, v))
    wq, wk, wv, wo = (np.asarray(x, np.float32) for x in (wq, wk, wv, wo))
    bq, bk, bv, bo = (np.asarray(x, np.float32) for x in (bq, bk, bv, bo))

    if "nc" not in _cache:
        _cache["nc"] = build_nc()
    nc = _cache["nc"]

    in_maps, bo_eff = prepare_core_inputs(q, k, v, wq, wk, wv, wo, bq, bk, bv, bo)
    res = run_bass_kernel_spmd(nc, in_maps, list(range(8)))
    _cache["last_results"] = res

    out = np.empty((B, S, D), np.float32)
    for b in range(B):
        out[b] = (res.results[2 * b]["out"].astype(np.float32)
                  + res.results[2 * b + 1]["out"].astype(np.float32) + bo_eff)
    return out
